# revision 13
# baseline (speedup 1.0000x reference)
"""MoE linear kernel for Trainium2, 8 NeuronCores, data-parallel over batch.

Problem (hardcoded shapes):
  x  [8192, 1024] f32, Wg [1024, 16], bg [16], We [16, 1024, 1024], be [16, 1024]
  out[b, o] = sum_e softmax(x @ Wg + bg)[b, e] * (x @ We[e] + be[e])[b, o]

Strategy: data-parallel over batch (1024 rows/core), no collectives.
Host pre-packs inputs into the exact SBUF layouts (transposed x, per-expert
weight tiles) so every DMA is contiguous and the PE does only matmuls:
  - gate logits via fp32r matmuls from xT, softmax on DVE/ACT,
    gate^T via one PE transpose per batch tile;
  - bias term (gate @ be) seeds each output accumulator via a K=16 matmul;
  - expert loop streams packed We once; per (expert, batch-tile) the two
    512-wide output halves are computed in interleaved PSUM groups so
    consecutive matmuls reuse the stationary operand (cheaper LDWEIGHTS);
  - DVE folds each group into the accumulator with one fused
    scalar_tensor_tensor: acc = psum * gate[:, e] + acc.

Expert matmuls run in fp32r (TF32-like, ~1.5e-4 matmul rel err) or bf16
(~2.4e-3) per MM_DTYPE; the gate path is always fp32r.
"""

import numpy as np

import concourse.bass as bass
import concourse.mybir as mybir
import concourse.tile as tile
from concourse.bass_utils import run_bass_kernel_spmd
from concourse.masks import make_identity

P = 128
B, D_IN, D_OUT, E = 8192, 1024, 1024, 16
NCORES = 8
BSH = B // NCORES          # 1024 batch rows per core
BT = BSH // P              # 8 batch tiles per core
KC = D_IN // P             # 8 contraction chunks
OH = 2                     # output halves
ON = D_OUT // OH           # 512 output cols per matmul group

F32 = mybir.dt.float32
F32R = mybir.dt.float32r
BF16 = mybir.dt.bfloat16

MM_DTYPE = "f32r"          # "f32r" | "bf16" for the expert matmuls


def _split_multi_waits(nc, limit=1):
    """The walrus build in this container rejects instructions carrying more
    than `limit` semaphore waits ("Too many sync wait commands" on the Tile
    tail drain). Move extra waits onto preceding same-engine NoOps."""
    n = 0
    for f in nc.m.functions:
        for bb in f.blocks:
            insts = bb.instructions
            i = 0
            while i < len(insts):
                ins = insts[i]
                si = ins.sync_info
                if si is not None and len(si.on_wait) > limit:
                    waits = list(si.on_wait)
                    extra, keep = waits[:-limit], waits[-limit:]
                    for j in range(0, len(extra), limit):
                        nop = mybir.InstNoOp(
                            name=f"I-waitsplit-{n}",
                            engine=ins.engine,
                            sync_info=mybir.SyncInfo(
                                on_wait=list(extra[j : j + limit]), on_update=[]
                            ),
                        )
                        n += 1
                        insts.insert(i, nop)
                        i += 1
                    si.on_wait = keep
                i += 1
    return n


def _emit_body(nc, pools, dram, mm_dtype):
    persist, we_pool, sm_pool, psum, gpsum = pools
    xtg, xte, wg, bgb, wep, be, out = dram
    EDT = F32R if mm_dtype == "f32r" else BF16

    ident = persist.tile([P, P], F32, tag="ident", name="ident")
    make_identity(nc, ident[:])

    # Small replicated tensors
    wg_s = persist.tile([P, KC, E], F32R, tag="wg", name="wg_s")
    nc.sync.dma_start(wg_s[:], wg.rearrange("(kc p) e -> p kc e", p=P).bitcast(F32R))
    bg_s = persist.tile([P, E], F32, tag="bg", name="bg_s")
    nc.sync.dma_start(bg_s[:], bgb[:])
    be_s = persist.tile([E, D_OUT], F32R, tag="be", name="be_s")
    nc.sync.dma_start(be_s[:], be[:].bitcast(F32R))

    # Transposed activations (pre-packed on host): [P, KC, BSH]
    xg = persist.tile([P, KC, BSH], F32R, tag="xg", name="xg")
    nc.sync.dma_start(xg[:], xtg[:].bitcast(F32R))
    if mm_dtype == "f32r":
        xe = xg
    else:
        xe = persist.tile([P, KC, BSH], EDT, tag="xe", name="xe")
        nc.sync.dma_start(xe[:], xte[:])

    gate = [
        persist.tile([P, E], F32, tag=f"g{bt}", name=f"g{bt}") for bt in range(BT)
    ]
    gateT = persist.tile([E, BSH], F32R, tag="gateT", name="gateT")
    acc = [
        [
            persist.tile([P, ON], F32, tag=f"acc{bt}_{oh}", name=f"acc{bt}_{oh}")
            for oh in range(OH)
        ]
        for bt in range(BT)
    ]

    # ---- Phase A: gate logits + softmax + gate^T ----
    for bt in range(BT):
        bsl = slice(bt * P, (bt + 1) * P)
        pg = gpsum.tile([P, E], F32, tag="pg", name="pg")
        for kc in range(KC):
            nc.tensor.matmul(
                pg[:],
                xg[:, kc, bsl],
                wg_s[:, kc, :],
                start=(kc == 0),
                stop=(kc == KC - 1),
            )
        logits = sm_pool.tile([P, E], F32, tag="logits", name="logits")
        nc.vector.tensor_add(logits[:], pg[:], bg_s[:])
        negmax = sm_pool.tile([P, 1], F32, tag="negmax", name="negmax")
        nc.vector.tensor_reduce(
            out=negmax[:],
            in_=logits[:],
            op=mybir.AluOpType.max,
            axis=mybir.AxisListType.X,
            negate=True,
        )
        esum = sm_pool.tile([P, 1], F32, tag="esum", name="esum")
        nc.scalar.activation(
            gate[bt][:],
            logits[:],
            mybir.ActivationFunctionType.Exp,
            bias=negmax[:, 0:1],
            accum_out=esum[:, 0:1],
        )
        rsum = sm_pool.tile([P, 1], F32, tag="rsum", name="rsum")
        nc.vector.reciprocal(rsum[:], esum[:])
        nc.vector.tensor_scalar_mul(gate[bt][:], gate[bt][:], rsum[:, 0:1])

        gtp = gpsum.tile([E, P], F32, tag="gtp", name="gtp")
        nc.tensor.transpose(gtp[:], gate[bt][:], ident[:])
        nc.vector.tensor_copy(gateT[:, bsl], gtp[:])

    # ---- Phase A.5: seed accumulators with gate @ be ----
    for bt in range(BT):
        for oh in range(OH):
            psb = psum.tile([P, ON], F32, tag="ps", name="psb")
            nc.tensor.matmul(
                psb[:],
                gateT[:, bt * P : (bt + 1) * P],
                be_s[:, oh * ON : (oh + 1) * ON],
                start=True,
                stop=True,
            )
            nc.vector.tensor_copy(acc[bt][oh][:], psb[:])

    # ---- Phase B: expert loop (packed We streamed once) ----
    for e in range(E):
        wt = we_pool.tile([P, KC, D_OUT], EDT, tag="we", name="wt")
        src = wep[e]
        nc.sync.dma_start(wt[:], src.bitcast(F32R) if mm_dtype == "f32r" else src)
        for bt in range(BT):
            bsl = slice(bt * P, (bt + 1) * P)
            ps = [
                psum.tile([P, ON], F32, tag="ps", name=f"ps{oh}") for oh in range(OH)
            ]
            for kc in range(KC):
                for oh in range(OH):
                    # consecutive oh-pair shares the stationary operand
                    nc.tensor.matmul(
                        ps[oh][:],
                        xe[:, kc, bsl],
                        wt[:, kc, oh * ON : (oh + 1) * ON],
                        start=(kc == 0),
                        stop=(kc == KC - 1),
                    )
            for oh in range(OH):
                nc.vector.scalar_tensor_tensor(
                    out=acc[bt][oh][:],
                    in0=ps[oh][:],
                    scalar=gate[bt][:, e : e + 1],
                    in1=acc[bt][oh][:],
                    op0=mybir.AluOpType.mult,
                    op1=mybir.AluOpType.add,
                )

    # ---- Phase C: store ----
    for bt in range(BT):
        for oh in range(OH):
            nc.sync.dma_start(
                out[bt * P : (bt + 1) * P, oh * ON : (oh + 1) * ON],
                acc[bt][oh][:],
            )


def _build(repeat=1, loop_n=1, mm_dtype=MM_DTYPE, psum_bufs=6, we_bufs=3):
    nc = bass.Bass(trn_type="TRN2")
    EDT = F32R if mm_dtype == "f32r" else BF16

    # Host-packed inputs (see make_in_maps):
    #   xtg: x shard transposed  [P, KC, BSH] f32 (gate path, fp32r view)
    #   xte: same in bf16 (expert path; only uploaded for bf16 variant)
    #   wep: We packed [E, P, KC, D_OUT] in expert dtype
    xtg = nc.dram_tensor("xtg", [P, KC, BSH], F32, kind="ExternalInput")
    xte = (
        nc.dram_tensor("xte", [P, KC, BSH], BF16, kind="ExternalInput")
        if mm_dtype == "bf16"
        else None
    )
    wg = nc.dram_tensor("wg", [D_IN, E], F32, kind="ExternalInput")
    bgb = nc.dram_tensor("bgb", [P, E], F32, kind="ExternalInput")
    wep = nc.dram_tensor(
        "wep", [E, P, KC, D_OUT], F32 if mm_dtype == "f32r" else BF16,
        kind="ExternalInput",
    )
    be = nc.dram_tensor("be", [E, D_OUT], F32, kind="ExternalInput")
    out = nc.dram_tensor("out", [BSH, D_OUT], F32, kind="ExternalOutput")
    dram = (xtg, xte, wg, bgb, wep, be, out)

    with tile.TileContext(nc) as tc:
        with (
            tc.tile_pool(name="persist", bufs=1) as persist,
            tc.tile_pool(name="wes", bufs=we_bufs) as we_pool,
            tc.tile_pool(name="sm", bufs=2) as sm_pool,
            tc.tile_pool(name="psum", bufs=psum_bufs, space="PSUM") as psum,
            tc.tile_pool(name="gpsum", bufs=1, space="PSUM") as gpsum,
        ):
            pools = (persist, we_pool, sm_pool, psum, gpsum)
            if loop_n > 1:
                with tc.For_i(0, loop_n, 1):
                    _emit_body(nc, pools, dram, mm_dtype)
            else:
                for _ in range(repeat):
                    _emit_body(nc, pools, dram, mm_dtype)

    _split_multi_waits(nc)
    return nc


_CACHE = {}


def _get_nc(repeat=1, **kw):
    key = ("nc", repeat, tuple(sorted(kw.items())))
    if key not in _CACHE:
        _CACHE[key] = _build(repeat, **kw)
    return _CACHE[key]


def make_in_maps(x, Wg, bg, We, be, mm_dtype=MM_DTYPE):
    import ml_dtypes

    x = np.ascontiguousarray(np.asarray(x, dtype=np.float32))
    Wg = np.ascontiguousarray(np.asarray(Wg, dtype=np.float32))
    bg = np.asarray(bg, dtype=np.float32).reshape(E)
    We = np.ascontiguousarray(np.asarray(We, dtype=np.float32))
    be = np.ascontiguousarray(np.asarray(be, dtype=np.float32))
    bgb = np.ascontiguousarray(np.broadcast_to(bg[None, :], (P, E)))

    # We packed to [E, P, KC, D_OUT]: wep[e, p, kc, o] = We[e, kc*P+p, o]
    wep = np.ascontiguousarray(
        We.reshape(E, KC, P, D_OUT).transpose(0, 2, 1, 3)
    )
    if mm_dtype == "bf16":
        wep = wep.astype(ml_dtypes.bfloat16)

    in_maps = []
    for c in range(NCORES):
        xs = x[c * BSH : (c + 1) * BSH]
        # xT packed to [P, KC, BSH]: xt[p, kc, b] = xs[b, kc*P+p]
        xt = np.ascontiguousarray(xs.reshape(BSH, KC, P).transpose(2, 1, 0))
        m = {"xtg": xt, "wg": Wg, "bgb": bgb, "wep": wep, "be": be}
        if mm_dtype == "bf16":
            m["xte"] = xt.astype(ml_dtypes.bfloat16)
        in_maps.append(m)
    return in_maps


def kernel(x, Wg, bg, We, be):
    nc = _get_nc(mm_dtype=MM_DTYPE)
    in_maps = make_in_maps(x, Wg, bg, We, be, mm_dtype=MM_DTYPE)
    res = run_bass_kernel_spmd(nc, in_maps, core_ids=list(range(NCORES)))
    return np.concatenate([r["out"] for r in res.results], axis=0)


# revision 14
# speedup vs baseline: 1.0444x; 1.0444x over previous
"""MoE linear kernel for Trainium2, 8 NeuronCores, data-parallel over batch.

Problem (hardcoded shapes):
  x  [8192, 1024] f32, Wg [1024, 16], bg [16], We [16, 1024, 1024], be [16, 1024]
  out[b, o] = sum_e softmax(x @ Wg + bg)[b, e] * (x @ We[e] + be[e])[b, o]

Strategy: data-parallel over batch (1024 rows/core), no collectives.
Host pre-packs inputs into the exact SBUF layouts (transposed x, per-expert
weight tiles) so every DMA is contiguous and the PE does only matmuls:
  - gate logits via fp32r matmuls from xT, softmax on DVE/ACT,
    gate^T via one PE transpose per batch tile;
  - bias term (gate @ be) seeds each output accumulator via a K=16 matmul;
  - expert loop streams packed We once; per (expert, batch-tile) the two
    512-wide output halves are computed in interleaved PSUM groups so
    consecutive matmuls reuse the stationary operand (cheaper LDWEIGHTS);
  - DVE folds each group into the accumulator with one fused
    scalar_tensor_tensor: acc = psum * gate[:, e] + acc.

Expert matmuls run in fp32r (TF32-like, ~1.5e-4 matmul rel err) or bf16
(~2.4e-3) per MM_DTYPE; the gate path is always fp32r.
"""

import numpy as np

import concourse.bass as bass
import concourse.mybir as mybir
import concourse.tile as tile
from concourse.bass_utils import run_bass_kernel_spmd
from concourse.masks import make_identity

P = 128
B, D_IN, D_OUT, E = 8192, 1024, 1024, 16
NCORES = 8
BSH = B // NCORES          # 1024 batch rows per core
BT = BSH // P              # 8 batch tiles per core
KC = D_IN // P             # 8 contraction chunks
OH = 2                     # output halves
ON = D_OUT // OH           # 512 output cols per matmul group

F32 = mybir.dt.float32
F32R = mybir.dt.float32r
BF16 = mybir.dt.bfloat16

MM_DTYPE = "f32r"          # "f32r" | "bf16" for the expert matmuls


def _split_multi_waits(nc, limit=1):
    """The walrus build in this container rejects instructions carrying more
    than `limit` semaphore waits ("Too many sync wait commands" on the Tile
    tail drain). Move extra waits onto preceding same-engine NoOps."""
    n = 0
    for f in nc.m.functions:
        for bb in f.blocks:
            insts = bb.instructions
            i = 0
            while i < len(insts):
                ins = insts[i]
                si = ins.sync_info
                if si is not None and len(si.on_wait) > limit:
                    waits = list(si.on_wait)
                    extra, keep = waits[:-limit], waits[-limit:]
                    for j in range(0, len(extra), limit):
                        nop = mybir.InstNoOp(
                            name=f"I-waitsplit-{n}",
                            engine=ins.engine,
                            sync_info=mybir.SyncInfo(
                                on_wait=list(extra[j : j + limit]), on_update=[]
                            ),
                        )
                        n += 1
                        insts.insert(i, nop)
                        i += 1
                    si.on_wait = keep
                i += 1
    return n


def _emit_body(nc, pools, dram, mm_dtype):
    persist, we_pool, sm_pool, psum, gpsum = pools
    xtg, xte, wg, bgb, wep, be, out = dram
    EDT = F32R if mm_dtype == "f32r" else BF16

    ident = persist.tile([P, P], F32, tag="ident", name="ident")
    make_identity(nc, ident[:])

    # Small replicated tensors
    wg_s = persist.tile([P, KC, E], F32R, tag="wg", name="wg_s")
    nc.sync.dma_start(wg_s[:], wg.rearrange("(kc p) e -> p kc e", p=P).bitcast(F32R))
    bg_s = persist.tile([P, E], F32, tag="bg", name="bg_s")
    nc.sync.dma_start(bg_s[:], bgb[:])
    be_s = persist.tile([E, D_OUT], F32R, tag="be", name="be_s")
    nc.sync.dma_start(be_s[:], be[:].bitcast(F32R))

    # Transposed activations (pre-packed on host): [P, KC, BSH]
    xg = persist.tile([P, KC, BSH], F32R, tag="xg", name="xg")
    nc.scalar.dma_start(xg[:], xtg[:].bitcast(F32R))
    if mm_dtype == "f32r":
        xe = xg
    else:
        xe = persist.tile([P, KC, BSH], EDT, tag="xe", name="xe")
        nc.sync.dma_start(xe[:], xte[:])

    gate = [
        persist.tile([P, E], F32, tag=f"g{bt}", name=f"g{bt}") for bt in range(BT)
    ]
    gateT = persist.tile([E, BSH], F32R, tag="gateT", name="gateT")
    acc = [
        [
            persist.tile([P, ON], F32, tag=f"acc{bt}_{oh}", name=f"acc{bt}_{oh}")
            for oh in range(OH)
        ]
        for bt in range(BT)
    ]

    # ---- Phase A: gate logits + softmax + gate^T ----
    for bt in range(BT):
        bsl = slice(bt * P, (bt + 1) * P)
        pg = gpsum.tile([P, E], F32, tag="pg", name="pg")
        for kc in range(KC):
            nc.tensor.matmul(
                pg[:],
                xg[:, kc, bsl],
                wg_s[:, kc, :],
                start=(kc == 0),
                stop=(kc == KC - 1),
            )
        logits = sm_pool.tile([P, E], F32, tag="logits", name="logits")
        nc.vector.tensor_add(logits[:], pg[:], bg_s[:])
        negmax = sm_pool.tile([P, 1], F32, tag="negmax", name="negmax")
        nc.vector.tensor_reduce(
            out=negmax[:],
            in_=logits[:],
            op=mybir.AluOpType.max,
            axis=mybir.AxisListType.X,
            negate=True,
        )
        esum = sm_pool.tile([P, 1], F32, tag="esum", name="esum")
        nc.scalar.activation(
            gate[bt][:],
            logits[:],
            mybir.ActivationFunctionType.Exp,
            bias=negmax[:, 0:1],
            accum_out=esum[:, 0:1],
        )
        rsum = sm_pool.tile([P, 1], F32, tag="rsum", name="rsum")
        nc.vector.reciprocal(rsum[:], esum[:])
        nc.vector.tensor_scalar_mul(gate[bt][:], gate[bt][:], rsum[:, 0:1])

        gtp = gpsum.tile([E, P], F32, tag="gtp", name="gtp")
        nc.tensor.transpose(gtp[:], gate[bt][:], ident[:])
        nc.vector.tensor_copy(gateT[:, bsl], gtp[:])

    # ---- Phase A.5: seed accumulators with gate @ be ----
    for bt in range(BT):
        for oh in range(OH):
            psb = psum.tile([P, ON], F32, tag="ps", name="psb")
            nc.tensor.matmul(
                psb[:],
                gateT[:, bt * P : (bt + 1) * P],
                be_s[:, oh * ON : (oh + 1) * ON],
                start=True,
                stop=True,
            )
            nc.vector.tensor_copy(acc[bt][oh][:], psb[:])

    # ---- Phase B: expert loop (packed We streamed once) ----
    for e in range(E):
        wt = we_pool.tile([P, KC, D_OUT], EDT, tag="we", name="wt")
        src = wep[e]
        nc.sync.dma_start(wt[:], src.bitcast(F32R) if mm_dtype == "f32r" else src)
        for bt in range(BT):
            bsl = slice(bt * P, (bt + 1) * P)
            ps = [
                psum.tile([P, ON], F32, tag="ps", name=f"ps{oh}") for oh in range(OH)
            ]
            for kc in range(KC):
                for oh in range(OH):
                    # consecutive oh-pair shares the stationary operand
                    nc.tensor.matmul(
                        ps[oh][:],
                        xe[:, kc, bsl],
                        wt[:, kc, oh * ON : (oh + 1) * ON],
                        start=(kc == 0),
                        stop=(kc == KC - 1),
                    )
            for oh in range(OH):
                nc.vector.scalar_tensor_tensor(
                    out=acc[bt][oh][:],
                    in0=ps[oh][:],
                    scalar=gate[bt][:, e : e + 1],
                    in1=acc[bt][oh][:],
                    op0=mybir.AluOpType.mult,
                    op1=mybir.AluOpType.add,
                )

    # ---- Phase C: store ----
    for bt in range(BT):
        for oh in range(OH):
            nc.sync.dma_start(
                out[bt * P : (bt + 1) * P, oh * ON : (oh + 1) * ON],
                acc[bt][oh][:],
            )


def _build(repeat=1, loop_n=1, mm_dtype=MM_DTYPE, psum_bufs=6, we_bufs=3):
    nc = bass.Bass(trn_type="TRN2")
    EDT = F32R if mm_dtype == "f32r" else BF16

    # Host-packed inputs (see make_in_maps):
    #   xtg: x shard transposed  [P, KC, BSH] f32 (gate path, fp32r view)
    #   xte: same in bf16 (expert path; only uploaded for bf16 variant)
    #   wep: We packed [E, P, KC, D_OUT] in expert dtype
    xtg = nc.dram_tensor("xtg", [P, KC, BSH], F32, kind="ExternalInput")
    xte = (
        nc.dram_tensor("xte", [P, KC, BSH], BF16, kind="ExternalInput")
        if mm_dtype == "bf16"
        else None
    )
    wg = nc.dram_tensor("wg", [D_IN, E], F32, kind="ExternalInput")
    bgb = nc.dram_tensor("bgb", [P, E], F32, kind="ExternalInput")
    wep = nc.dram_tensor(
        "wep", [E, P, KC, D_OUT], F32 if mm_dtype == "f32r" else BF16,
        kind="ExternalInput",
    )
    be = nc.dram_tensor("be", [E, D_OUT], F32, kind="ExternalInput")
    out = nc.dram_tensor("out", [BSH, D_OUT], F32, kind="ExternalOutput")
    dram = (xtg, xte, wg, bgb, wep, be, out)

    with tile.TileContext(nc) as tc:
        with (
            tc.tile_pool(name="persist", bufs=1) as persist,
            tc.tile_pool(name="wes", bufs=we_bufs) as we_pool,
            tc.tile_pool(name="sm", bufs=2) as sm_pool,
            tc.tile_pool(name="psum", bufs=psum_bufs, space="PSUM") as psum,
            tc.tile_pool(name="gpsum", bufs=1, space="PSUM") as gpsum,
        ):
            pools = (persist, we_pool, sm_pool, psum, gpsum)
            if loop_n > 1:
                with tc.For_i(0, loop_n, 1):
                    _emit_body(nc, pools, dram, mm_dtype)
            else:
                for _ in range(repeat):
                    _emit_body(nc, pools, dram, mm_dtype)

    _split_multi_waits(nc)
    return nc


_CACHE = {}


def _get_nc(repeat=1, **kw):
    key = ("nc", repeat, tuple(sorted(kw.items())))
    if key not in _CACHE:
        _CACHE[key] = _build(repeat, **kw)
    return _CACHE[key]


def make_in_maps(x, Wg, bg, We, be, mm_dtype=MM_DTYPE):
    import ml_dtypes

    x = np.ascontiguousarray(np.asarray(x, dtype=np.float32))
    Wg = np.ascontiguousarray(np.asarray(Wg, dtype=np.float32))
    bg = np.asarray(bg, dtype=np.float32).reshape(E)
    We = np.ascontiguousarray(np.asarray(We, dtype=np.float32))
    be = np.ascontiguousarray(np.asarray(be, dtype=np.float32))
    bgb = np.ascontiguousarray(np.broadcast_to(bg[None, :], (P, E)))

    # We packed to [E, P, KC, D_OUT]: wep[e, p, kc, o] = We[e, kc*P+p, o]
    wep = np.ascontiguousarray(
        We.reshape(E, KC, P, D_OUT).transpose(0, 2, 1, 3)
    )
    if mm_dtype == "bf16":
        wep = wep.astype(ml_dtypes.bfloat16)

    in_maps = []
    for c in range(NCORES):
        xs = x[c * BSH : (c + 1) * BSH]
        # xT packed to [P, KC, BSH]: xt[p, kc, b] = xs[b, kc*P+p]
        xt = np.ascontiguousarray(xs.reshape(BSH, KC, P).transpose(2, 1, 0))
        m = {"xtg": xt, "wg": Wg, "bgb": bgb, "wep": wep, "be": be}
        if mm_dtype == "bf16":
            m["xte"] = xt.astype(ml_dtypes.bfloat16)
        in_maps.append(m)
    return in_maps


def kernel(x, Wg, bg, We, be):
    nc = _get_nc(mm_dtype=MM_DTYPE)
    in_maps = make_in_maps(x, Wg, bg, We, be, mm_dtype=MM_DTYPE)
    res = run_bass_kernel_spmd(nc, in_maps, core_ids=list(range(NCORES)))
    return np.concatenate([r["out"] for r in res.results], axis=0)
